# revision 1
# baseline (speedup 1.0000x reference)
"""Trainium2 Bass kernel: dark-channel + 15x15 erosion (min-pool, stride 1,
+inf padding), data-parallel over 8 NeuronCores.

Input  I: [32, 3, 512, 512] f32, k: scalar (15)
Output:   [32, 1, 512, 512] f32  (min over channels, then kxk spatial min)

Per-core plan (4 images each, pipelined via Tile pools):
  1. DMA the image (3 channels, one transfer) into SBUF, rows on partitions.
  2. Channel min on DVE (2 f32 tensor_tensor min ops, second in place).
  3. Horizontal 15-min-filter on DVE: dyadic shifted mins (1,2,4,7); the
     first stage also converts f32 -> f16.
  4. PE transpose (identity matmul), 4 blocks per PSUM bank, one ScalarE
     evac per bank -> column layout.
  5. Vertical 15-min-filter on DVE (same dyadic trick along free dim).
  6. PE transpose back + ScalarE evac (f16 -> f32 cast) -> row layout.
  7. DMA result to HBM.

fp16 intermediates: values are mins of uniform[0,1) data; min is selection,
not arithmetic, so fp16 keeps rel err ~1e-4.  Pad value 30000.0 acts as
+inf for this data range.

The walrus backend encodes at most ONE sync-wait per instruction and fails
codegen with "Too many sync wait commands" otherwise, while Tile freely
emits several (pool slot reuse, kernel-tail drain).  The post-pass at the
end of _build_nc hoists all but one wait of every instruction onto
single-wait NOPs inserted right before it on the same engine - identical
semantics (the engine sequencer performs the waits in order), and every
instruction then fits the encoding.  CoreSim cannot execute the inserted
NOPs, so the simulator path builds with split_waits=False.
"""

import sys

if "/opt/trn_rl_repo" not in sys.path:
    sys.path.insert(0, "/opt/trn_rl_repo")

import numpy as np

N_CORES = 8
IMGS = 4          # images per core
C = 3
H = W = 512
K = 15
PAD = K // 2      # 7
L = 8             # left pad in filter buffers (>= PAD+1, power of 2)
PITCH = L + 512 + 8   # 528, padded row/col length
NJ = H // 128     # row tiles
NB = W // 128     # col blocks
JH = NJ // 2      # row tiles per half-image
PADV = 30000.0    # effective +inf for data in [0,1)

_cache = {}


def _build_nc(use_f16=True, split_waits=True, io_bufs=4,
              work_bufs=3, res_bufs=6, out_bufs=2, store_per_j=True,
              conv_c2=True, out_on_act=False):
    import concourse.bass as bass
    import concourse.mybir as mybir
    import concourse.tile as tile
    import concourse.masks as masks

    F32 = mybir.dt.float32
    FI = mybir.dt.float16 if use_f16 else F32
    MIN = mybir.AluOpType.min

    nc = bass.Bass("TRN2", target_bir_lowering=False, debug=False)
    inp = nc.dram_tensor("inp", [IMGS, C, H, W], F32, kind="ExternalInput")
    out = nc.dram_tensor("out", [IMGS, 1, H, W], F32, kind="ExternalOutput")

    def dyadic(pool, src, n):
        """15-wide min filter along last dim of src [128, n, PITCH];
        logical x at [L : L+512].  Returns [128, n, 512] f16.  The first
        stage converts src's dtype (f32 for the h-pass) to f16."""
        f2 = pool.tile([128, n, PITCH], FI, tag="fa", name="f2")
        nc.vector.tensor_tensor(
            f2[:, :, 0:526], src[:, :, 0:526], src[:, :, 1:527], op=MIN
        )
        f4 = pool.tile([128, n, PITCH], FI, tag="fb", name="f4")
        nc.vector.tensor_tensor(
            f4[:, :, 0:524], f2[:, :, 0:524], f2[:, :, 2:526], op=MIN
        )
        f8 = pool.tile([128, n, PITCH], FI, tag="fa", name="f8")
        nc.vector.tensor_tensor(
            f8[:, :, 0:520], f4[:, :, 0:520], f4[:, :, 4:524], op=MIN
        )
        res = pool.tile([128, n, 512], FI, tag="res", name="res")
        nc.vector.tensor_tensor(
            res[:], f8[:, :, 1:513], f8[:, :, 8:520], op=MIN
        )
        return res

    with tile.TileContext(nc) as tc:
        with (
            tc.tile_pool(name="const", bufs=1) as cpool,
            tc.tile_pool(name="io", bufs=io_bufs) as io_pool,
            tc.tile_pool(name="work", bufs=work_bufs) as work,
            tc.tile_pool(name="resw", bufs=res_bufs) as resw,
            tc.tile_pool(name="opool", bufs=out_bufs) as opool,
            tc.tile_pool(name="psum", bufs=8, space="PSUM") as psum,
        ):
            ident = cpool.tile([128, 128], FI)
            masks.make_identity(nc, ident[:])

            def stage1(i):
                # --- per half-image: 3 per-channel DMAs (0.5 MB each) so
                # the filter chain starts after ~1/6 of the image loads;
                # the multi-semaphore waits this creates are legalized by
                # the wait-splitting post-pass.  The otherwise-idle
                # ScalarE converts each channel to f16 as soon as its DMA
                # lands, so every DVE op downstream runs in the 2x_1P
                # perf mode (f32 operands cap tensor_tensor at 1x).
                r_halves = []
                for hh in range(2):
                    in_h = io_pool.tile([128, C, JH, W], F32, tag="in_t",
                                        name="in_h")
                    in16 = io_pool.tile([128, C, JH, W], FI, tag="in16",
                                        name="in16")
                    for c in range(C):
                        nc.sync.dma_start(
                            in_h[:, c],
                            inp[i, c, 256 * hh : 256 * (hh + 1)].rearrange(
                                "(j p) w -> p j w", p=128
                            ),
                        )
                        if c < 2 or conv_c2:
                            nc.scalar.copy(in16[:, c], in_h[:, c])
                    # channel min (DVE, all f16; walrus rejects
                    # tensor_tensor on Pool), second min in place.
                    xpad = work.tile([128, JH, PITCH], FI, tag="xp",
                                     name="xpad")
                    nc.gpsimd.memset(xpad[:, :, 0:L], PADV)
                    nc.gpsimd.memset(xpad[:, :, L + W : PITCH], PADV)
                    nc.vector.tensor_tensor(
                        xpad[:, :, L : L + W], in16[:, 0, :, :],
                        in16[:, 1, :, :], op=MIN
                    )
                    nc.vector.tensor_tensor(
                        xpad[:, :, L : L + W], xpad[:, :, L : L + W],
                        in16[:, 2, :, :] if conv_c2 else in_h[:, 2, :, :],
                        op=MIN,
                    )
                    # horizontal filter (DVE, f16)
                    r_halves.append(dyadic(resw, xpad, JH))
                return r_halves

            def stage2(i, r_halves):
                # --- transpose to column layout; 4 blocks (all j for one
                # b) fill one PSUM bank, ONE ACT evac per bank.
                vb = work.tile([128, NB, PITCH], FI, tag="vb", name="vb")
                nc.gpsimd.memset(vb[:, :, 0:L], PADV)
                nc.gpsimd.memset(vb[:, :, L + H : PITCH], PADV)
                for b in range(NB):
                    pt = psum.tile([128, 2 * NJ, 128], FI, tag="pt",
                                   name="pt")
                    for j in range(NJ):
                        rh = r_halves[j // JH]
                        nc.tensor.transpose(
                            pt[:, j, :],
                            rh[:, j % JH, 128 * b : 128 * (b + 1)],
                            ident[:],
                        )
                    nc.scalar.copy(
                        vb[:, b, L : L + H],
                        pt[:, 0:NJ, :].rearrange("p n w -> p (n w)"),
                    )

                # --- vertical filter per column-block pair (finer grain:
                # back-transposes of pair 0 overlap pair 1's filter)
                u_pairs = [
                    dyadic(resw, vb[:, 2 * bp : 2 * (bp + 1), :], 2)
                    for bp in range(2)
                ]

                # --- transpose back, f32 out; store per half-image
                # (or per row-tile with store_per_j)
                o = opool.tile([128, NJ, W], F32, name="o")
                dma_eng = nc.scalar if out_on_act else nc.sync
                for hh in range(2):
                    for j in range(JH * hh, JH * (hh + 1)):
                        pt = psum.tile([128, 2 * NB, 128], FI, tag="pt",
                                       name="pt")
                        for b in range(NB):
                            nc.tensor.transpose(
                                pt[:, b, :],
                                u_pairs[b // 2][
                                    :, b % 2, 128 * j : 128 * (j + 1)
                                ],
                                ident[:],
                            )
                        nc.scalar.copy(
                            o[:, j, :],
                            pt[:, 0:NB, :].rearrange("p n w -> p (n w)"),
                        )
                        if store_per_j:
                            dma_eng.dma_start(
                                out[i, 0, 128 * j : 128 * (j + 1)].rearrange(
                                    "(q p) w -> p q w", p=128
                                ),
                                o[:, j : j + 1, :],
                            )
                    if not store_per_j:
                        dma_eng.dma_start(
                            out[i, 0, 256 * hh : 256 * (hh + 1)].rearrange(
                                "(j p) w -> p j w", p=128
                            ),
                            o[:, JH * hh : JH * (hh + 1), :],
                        )

            for i in range(IMGS):
                stage2(i, stage1(i))

    if not split_waits:
        return nc
    # Post-pass: walrus encodes at most ONE sync-wait per instruction.
    # Hoist all but one wait of any multi-wait instruction onto
    # single-wait NOPs inserted just before it on the same engine
    # (identical semantics: the sequencer performs the waits in order).
    nsplit = 0
    for bb in nc.main_func.blocks:
        idx = 0
        while idx < len(bb.instructions):
            ins = bb.instructions[idx]
            si = ins.sync_info
            if si is not None and si.on_wait and len(si.on_wait) > 1:
                waits = list(si.on_wait)
                for w in waits[:-1]:
                    nop = mybir.InstNoOp(
                        name=f"W-split-{nsplit}", ins=[], outs=[]
                    )
                    nop.engine = ins.engine
                    nop.sync_info = mybir.SyncInfo(
                        on_wait=[w], on_update=[]
                    )
                    bb.instructions.insert(idx, nop)
                    nsplit += 1
                    idx += 1
                ins.sync_info = mybir.SyncInfo(
                    on_wait=[waits[-1]], on_update=list(si.on_update or [])
                )
            idx += 1
    return nc


def _get_nc():
    if "nc" not in _cache:
        _cache["nc"] = _build_nc()
    return _cache["nc"]


def kernel(I, k):
    from concourse.bass_utils import run_bass_kernel_spmd

    k = int(np.asarray(k))
    assert k == K, f"kernel compiled for k={K}, got {k}"
    I = np.ascontiguousarray(np.asarray(I), dtype=np.float32)
    B = I.shape[0]
    assert I.shape == (B, C, H, W) and B == N_CORES * IMGS

    nc = _get_nc()
    in_maps = [
        {"inp": I[c * IMGS : (c + 1) * IMGS]} for c in range(N_CORES)
    ]
    res = run_bass_kernel_spmd(nc, in_maps, list(range(N_CORES))).results
    return np.concatenate([res[c]["out"] for c in range(N_CORES)], axis=0)



# revision 3
# speedup vs baseline: 1.0827x; 1.0827x over previous
"""Trainium2 Bass kernel v2: dark-channel + 15x15 erosion, data-parallel
over 8 NeuronCores.

Input  I: [32, 3, 512, 512] f32, k: scalar (15)
Output:   [32, 1, 512, 512] f32

v2 plan (4 images per core):
  1. The channel min ("dark channel") is computed BY THE DMA: gpsimd
     (SWDGE) DMAs can cast f32->f16 in flight and accumulate with
     AluOpType.min.  Three chained Pool DMAs per image land min(c0,c1,c2)
     as f16 directly into the padded H-filter buffer.  This removes the
     ScalarE f32->f16 conversions and the DVE channel-min entirely, and
     the cost model charges the (smaller) f16 SBUF side.
  2. Horizontal 15-min-filter on DVE: dyadic shifted mins, all f16 (2x).
  3. PE transpose (identity matmul) -> PSUM, ScalarE evac per 128-col
     block -> column layout.
  4. Vertical 15-min-filter on DVE per column-block pair.
  5. PE transpose back, ScalarE evac casts f16->f32, per-j DMA to HBM
     on SP.
  Optionally (OFF_H/OFF_V), individual filter stages are offloaded from
  DVE to a pair of DMAs: a plain shifted copy on a HWDGE queue plus a
  shifted min-accumulate on Pool.

fp16 intermediates: min is selection, not arithmetic; rel err ~1e-4 on
uniform[0,1) data.  Pad value 30000.0 acts as +inf.

The walrus backend encodes at most ONE sync-wait per instruction; the
post-pass at the end of _build_nc hoists extra waits onto single-wait
NOPs (identical semantics).  CoreSim can't execute the NOPs, so the sim
path builds with split_waits=False.
"""

import sys

if "/opt/trn_rl_repo" not in sys.path:
    sys.path.insert(0, "/opt/trn_rl_repo")

import numpy as np

N_CORES = 8
IMGS = 4          # images per core
C = 3
H = W = 512
K = 15
PAD = K // 2      # 7
L = 8             # left pad in filter buffers (>= PAD+1, power of 2)
PITCH = L + 512 + 8   # 528
NJ = H // 128     # row tiles
NB = W // 128     # col blocks
PADV = 30000.0    # effective +inf for data in [0,1)

_cache = {}

# Dyadic 15-tap min filter: (offset_a, offset_b, lo, hi) per stage, on a
# PITCH-padded buffer with logical x at [L, L+512).  Stage s output f_s
# defined on [lo, hi); final result = f[1:513] min f[8:520] -> [512].
STAGES = [
    (0, 1, 0, 526),
    (0, 2, 0, 524),
    (0, 4, 0, 520),
    (1, 8, 0, 512),   # res[w] = f8[w+1] min f8[w+8]
]


_OFF_H = ()
_EVAC_DVE = tuple((i, b) for i in range(IMGS) for b in (1, 3))


def _build_nc(split_waits=True, off_h=_OFF_H, off_v=(), work_bufs=4,
              res_bufs=6, out_bufs=2, psum_bufs=8, copy_eng="sync",
              par_load=(0, 1, 2, 3), act_warm=True, pl_bufs=3, two_chain=(),
              evac_dve=_EVAC_DVE, h_split=2, v_split=2):
    """off_h/off_v: iterable of (img, stage) pairs offloaded to DMA."""
    import concourse.bass as bass
    import concourse.mybir as mybir
    import concourse.tile as tile
    import concourse.masks as masks

    F32 = mybir.dt.float32
    FI = mybir.dt.float16
    MIN = mybir.AluOpType.min

    off_h = set(off_h)
    off_v = set(off_v)

    nc = bass.Bass("TRN2", target_bir_lowering=False, debug=False)
    inp = nc.dram_tensor("inp", [IMGS, C, H, W], F32, kind="ExternalInput")
    out = nc.dram_tensor("out", [IMGS, 1, H, W], F32, kind="ExternalOutput")

    def dyadic(pool, src, n, off=(), split_last=0):
        """15-wide min filter along last dim of src [128, n, PITCH].
        Returns [128, n, 512] f16.  Stages whose index is in `off` run as
        DMA copy (HWDGE) + min-accumulate (Pool SWDGE) instead of DVE.
        split_last=q emits the final stage as q chunks along the output
        columns so downstream transposes can start earlier."""
        cur = src
        copy_q = getattr(nc, "sync" if copy_eng == "sync" else "scalar")
        for s, (oa, ob, lo, hi) in enumerate(STAGES):
            last = s == len(STAGES) - 1
            shape = [128, n, 512 if last else PITCH]
            tag = "res" if last else ("fa" if s % 2 == 0 else "fb")
            nxt = pool.tile(shape, FI, tag=tag, name=f"f{s}")
            if last and split_last > 1 and s not in off:
                cw = 512 // split_last
                for q in range(split_last):
                    nc.vector.tensor_tensor(
                        nxt[:, :, q * cw : (q + 1) * cw],
                        cur[:, :, oa + q * cw : oa + (q + 1) * cw],
                        cur[:, :, ob + q * cw : ob + (q + 1) * cw], op=MIN,
                    )
                cur = nxt
                continue
            dst = nxt[:] if last else nxt[:, :, lo:hi]
            if s in off:
                copy_q.dma_start(dst, cur[:, :, oa + lo : oa + hi])
                nc.gpsimd.dma_start(
                    dst, cur[:, :, ob + lo : ob + hi], accum_op=MIN
                )
            else:
                nc.vector.tensor_tensor(
                    dst, cur[:, :, oa + lo : oa + hi],
                    cur[:, :, ob + lo : ob + hi], op=MIN,
                )
            cur = nxt
        return cur

    with tile.TileContext(nc) as tc:
        with (
            tc.tile_pool(name="const", bufs=1) as cpool,
            tc.tile_pool(name="work", bufs=work_bufs) as work,
            tc.tile_pool(name="resw", bufs=res_bufs) as resw,
            tc.tile_pool(name="opool", bufs=out_bufs) as opool,
            tc.tile_pool(name="plp", bufs=pl_bufs) as plp,
            tc.tile_pool(name="psum", bufs=psum_bufs, space="PSUM") as psum,
        ):
            ident = cpool.tile([128, 128], FI)
            masks.make_identity(nc, ident[:])
            if act_warm:
                # Touch ScalarE once so the activation-table load happens
                # during the DMA fill, not on the first PSUM evac.
                warm = cpool.tile([128, 1], FI)
                nc.scalar.copy(warm[:], ident[:, 0:1])

            def stage1(i):
                # Dark channel via Pool-DMA chain: cast f32->f16, then two
                # min-accumulates, directly into the padded filter buffer.
                xpad = work.tile([128, NJ, PITCH], FI, tag="xp", name="xpad")
                nc.gpsimd.memset(xpad[:, :, 0:L], PADV)
                nc.gpsimd.memset(xpad[:, :, L + W : PITCH], PADV)
                interior = xpad[:, :, L : L + W]
                src = lambda c: inp[i, c].rearrange("(j p) w -> p j w", p=128)
                if i in par_load:
                    # Low-latency variant for pipeline fill: 3 independent
                    # casting DMAs (no accum chain), channel min on DVE.
                    pb = [
                        plp.tile([128, NJ, W], FI, tag=f"pb{c}", name=f"pl{c}")
                        for c in range(C)
                    ]
                    for c in range(C):
                        nc.gpsimd.dma_start(pb[c][:], src(c))
                    nc.vector.tensor_tensor(interior, pb[0][:], pb[1][:],
                                            op=MIN)
                    nc.vector.tensor_tensor(interior, interior, pb[2][:],
                                            op=MIN)
                elif i in two_chain:
                    # 2-link accum chain + parallel third channel; one DVE
                    # merge.  Lower latency than the 3-link chain.
                    a = plp.tile([128, NJ, W], FI, tag="pb0", name="tc_a")
                    b = plp.tile([128, NJ, W], FI, tag="pb1", name="tc_b")
                    nc.gpsimd.dma_start(a[:], src(0))
                    nc.gpsimd.dma_start(b[:], src(2))
                    nc.gpsimd.dma_start(a[:], src(1), accum_op=MIN)
                    nc.vector.tensor_tensor(interior, a[:], b[:], op=MIN)
                else:
                    for c in range(C):
                        nc.gpsimd.dma_start(
                            interior, src(c),
                            accum_op=mybir.AluOpType.bypass if c == 0 else MIN,
                        )
                # horizontal filter
                return dyadic(resw, xpad, NJ,
                              off={s for (ii, s) in off_h if ii == i},
                              split_last=h_split)

            def stage2(i, r):
                # transpose to column layout; one PSUM bank + one ACT evac
                # per 128-column block.
                vb = work.tile([128, NB, PITCH], FI, tag="vb", name="vb")
                nc.gpsimd.memset(vb[:, :, 0:L], PADV)
                nc.gpsimd.memset(vb[:, :, L + H : PITCH], PADV)
                for b in range(NB):
                    pt = psum.tile([128, NJ, 128], FI, tag="pt", name="pt")
                    for j in range(NJ):
                        nc.tensor.transpose(
                            pt[:, j, :], r[:, j, 128 * b : 128 * (b + 1)],
                            ident[:],
                        )
                    src_f = pt[:, :, :].rearrange("p n w -> p (n w)")
                    if (i, b) in evac_dve:
                        nc.vector.tensor_copy(vb[:, b, L : L + H], src_f)
                    else:
                        nc.scalar.copy(vb[:, b, L : L + H], src_f)

                # vertical filter per column-block pair
                offv = {s for (ii, s) in off_v if ii == i}
                u_pairs = [
                    dyadic(resw, vb[:, 2 * bp : 2 * (bp + 1), :], 2, off=offv,
                           split_last=v_split)
                    for bp in range(2)
                ]

                # transpose back, f32 out, store per row-tile
                o = opool.tile([128, NJ, W], F32, name="o")
                for j in range(NJ):
                    pt = psum.tile([128, NB, 128], FI, tag="pt", name="pt2")
                    for b in range(NB):
                        nc.tensor.transpose(
                            pt[:, b, :],
                            u_pairs[b // 2][:, b % 2, 128 * j : 128 * (j + 1)],
                            ident[:],
                        )
                    nc.scalar.copy(
                        o[:, j, :],
                        pt[:, :, :].rearrange("p n w -> p (n w)"),
                    )
                    nc.sync.dma_start(
                        out[i, 0, 128 * j : 128 * (j + 1)].rearrange(
                            "(q p) w -> p q w", p=128
                        ),
                        o[:, j : j + 1, :],
                    )

            for i in range(IMGS):
                stage2(i, stage1(i))

    if not split_waits:
        return nc
    import concourse.mybir as mybir
    nsplit = 0
    for bb in nc.main_func.blocks:
        idx = 0
        while idx < len(bb.instructions):
            ins = bb.instructions[idx]
            si = ins.sync_info
            if si is not None and si.on_wait and len(si.on_wait) > 1:
                waits = list(si.on_wait)
                for w in waits[:-1]:
                    nop = mybir.InstNoOp(
                        name=f"W-split-{nsplit}", ins=[], outs=[]
                    )
                    nop.engine = ins.engine
                    nop.sync_info = mybir.SyncInfo(on_wait=[w], on_update=[])
                    bb.instructions.insert(idx, nop)
                    nsplit += 1
                    idx += 1
                ins.sync_info = mybir.SyncInfo(
                    on_wait=[waits[-1]], on_update=list(si.on_update or [])
                )
            idx += 1
    return nc


def _get_nc():
    if "nc" not in _cache:
        _cache["nc"] = _build_nc()
    return _cache["nc"]


def kernel(I, k):
    from concourse.bass_utils import run_bass_kernel_spmd

    k = int(np.asarray(k))
    assert k == K, f"kernel compiled for k={K}, got {k}"
    I = np.ascontiguousarray(np.asarray(I), dtype=np.float32)
    B = I.shape[0]
    assert I.shape == (B, C, H, W) and B == N_CORES * IMGS

    nc = _get_nc()
    in_maps = [
        {"inp": I[c * IMGS : (c + 1) * IMGS]} for c in range(N_CORES)
    ]
    res = run_bass_kernel_spmd(nc, in_maps, list(range(N_CORES))).results
    return np.concatenate([res[c]["out"] for c in range(N_CORES)], axis=0)


# revision 4
# speedup vs baseline: 1.1118x; 1.0270x over previous
"""Trainium2 Bass kernel v2: dark-channel + 15x15 erosion, data-parallel
over 8 NeuronCores.

Input  I: [32, 3, 512, 512] f32, k: scalar (15)
Output:   [32, 1, 512, 512] f32

v2 plan (4 images per core):
  1. The channel min ("dark channel") is computed BY THE DMA: gpsimd
     (SWDGE) DMAs can cast f32->f16 in flight and accumulate with
     AluOpType.min.  Three chained Pool DMAs per image land min(c0,c1,c2)
     as f16 directly into the padded H-filter buffer.  This removes the
     ScalarE f32->f16 conversions and the DVE channel-min entirely, and
     the cost model charges the (smaller) f16 SBUF side.
  2. Horizontal 15-min-filter on DVE: dyadic shifted mins, all f16 (2x).
  3. PE transpose (identity matmul) -> PSUM, ScalarE evac per 128-col
     block -> column layout.
  4. Vertical 15-min-filter on DVE per column-block pair.
  5. PE transpose back, ScalarE evac casts f16->f32, per-j DMA to HBM
     on SP.
  Optionally (OFF_H/OFF_V), individual filter stages are offloaded from
  DVE to a pair of DMAs: a plain shifted copy on a HWDGE queue plus a
  shifted min-accumulate on Pool.

fp16 intermediates: min is selection, not arithmetic; rel err ~1e-4 on
uniform[0,1) data.  Pad value 30000.0 acts as +inf.

The walrus backend encodes at most ONE sync-wait per instruction; the
post-pass at the end of _build_nc hoists extra waits onto single-wait
NOPs (identical semantics).  CoreSim can't execute the NOPs, so the sim
path builds with split_waits=False.
"""

import sys

if "/opt/trn_rl_repo" not in sys.path:
    sys.path.insert(0, "/opt/trn_rl_repo")

import numpy as np

N_CORES = 8
IMGS = 4          # images per core
C = 3
H = W = 512
K = 15
PAD = K // 2      # 7
L = 8             # left pad in filter buffers (>= PAD+1, power of 2)
PITCH = L + 512 + 8   # 528
NJ = H // 128     # row tiles
NB = W // 128     # col blocks
PADV = 30000.0    # effective +inf for data in [0,1)

_cache = {}

# Dyadic 15-tap min filter: (offset_a, offset_b, lo, hi) per stage, on a
# PITCH-padded buffer with logical x at [L, L+512).  Stage s output f_s
# defined on [lo, hi); final result = f[1:513] min f[8:520] -> [512].
STAGES = [
    (0, 1, 0, 526),
    (0, 2, 0, 524),
    (0, 4, 0, 520),
    (1, 8, 0, 512),   # res[w] = f8[w+1] min f8[w+8]
]


_OFF_H = ()
_EVAC_DVE = tuple((i, b) for i in range(IMGS) for b in (1, 3))


def _build_nc(split_waits=True, off_h=_OFF_H, off_v=(), work_bufs=4,
              res_bufs=6, out_bufs=2, psum_bufs=8, copy_eng="sync",
              par_load=(0, 1, 2, 3), act_warm=True, pl_bufs=3, two_chain=(),
              evac_dve=_EVAC_DVE, h_split=2, v_split=2, out_eng="sync",
              halves=True):
    """off_h/off_v: iterable of (img, stage) pairs offloaded to DMA."""
    import concourse.bass as bass
    import concourse.mybir as mybir
    import concourse.tile as tile
    import concourse.masks as masks

    F32 = mybir.dt.float32
    FI = mybir.dt.float16
    MIN = mybir.AluOpType.min

    off_h = set(off_h)
    off_v = set(off_v)

    nc = bass.Bass("TRN2", target_bir_lowering=False, debug=False)
    inp = nc.dram_tensor("inp", [IMGS, C, H, W], F32, kind="ExternalInput")
    out = nc.dram_tensor("out", [IMGS, 1, H, W], F32, kind="ExternalOutput")

    def dyadic(pool, src, n, off=(), split_last=0):
        """15-wide min filter along last dim of src [128, n, PITCH].
        Returns [128, n, 512] f16.  Stages whose index is in `off` run as
        DMA copy (HWDGE) + min-accumulate (Pool SWDGE) instead of DVE.
        split_last=q emits the final stage as q chunks along the output
        columns so downstream transposes can start earlier."""
        cur = src
        copy_q = getattr(nc, "sync" if copy_eng == "sync" else "scalar")
        for s, (oa, ob, lo, hi) in enumerate(STAGES):
            last = s == len(STAGES) - 1
            shape = [128, n, 512 if last else PITCH]
            tag = "res" if last else ("fa" if s % 2 == 0 else "fb")
            nxt = pool.tile(shape, FI, tag=tag, name=f"f{s}")
            if last and split_last > 1 and s not in off:
                cw = 512 // split_last
                for q in range(split_last):
                    nc.vector.tensor_tensor(
                        nxt[:, :, q * cw : (q + 1) * cw],
                        cur[:, :, oa + q * cw : oa + (q + 1) * cw],
                        cur[:, :, ob + q * cw : ob + (q + 1) * cw], op=MIN,
                    )
                cur = nxt
                continue
            dst = nxt[:] if last else nxt[:, :, lo:hi]
            if s in off:
                copy_q.dma_start(dst, cur[:, :, oa + lo : oa + hi])
                nc.gpsimd.dma_start(
                    dst, cur[:, :, ob + lo : ob + hi], accum_op=MIN
                )
            else:
                nc.vector.tensor_tensor(
                    dst, cur[:, :, oa + lo : oa + hi],
                    cur[:, :, ob + lo : ob + hi], op=MIN,
                )
            cur = nxt
        return cur

    with tile.TileContext(nc) as tc:
        with (
            tc.tile_pool(name="const", bufs=1) as cpool,
            tc.tile_pool(name="work", bufs=work_bufs) as work,
            tc.tile_pool(name="resw", bufs=res_bufs) as resw,
            tc.tile_pool(name="opool", bufs=out_bufs) as opool,
            tc.tile_pool(name="plp", bufs=pl_bufs) as plp,
            tc.tile_pool(name="psum", bufs=psum_bufs, space="PSUM") as psum,
        ):
            ident = cpool.tile([128, 128], FI)
            masks.make_identity(nc, ident[:])
            if act_warm:
                # Touch ScalarE once so the activation-table load happens
                # during the DMA fill, not on the first PSUM evac.
                warm = cpool.tile([128, 1], FI)
                nc.scalar.copy(warm[:], ident[:, 0:1])

            def stage1(i):
                # Dark channel via Pool-DMA chain: cast f32->f16, then two
                # min-accumulates, directly into the padded filter buffer.
                xpad = work.tile([128, NJ, PITCH], FI, tag="xp", name="xpad")
                nc.gpsimd.memset(xpad[:, :, 0:L], PADV)
                nc.gpsimd.memset(xpad[:, :, L + W : PITCH], PADV)
                interior = xpad[:, :, L : L + W]
                src = lambda c: inp[i, c].rearrange("(j p) w -> p j w", p=128)
                if i in par_load:
                    # Low-latency variant for pipeline fill: 3 independent
                    # casting DMAs (no accum chain), channel min on DVE.
                    pb = [
                        plp.tile([128, NJ, W], FI, tag=f"pb{c}", name=f"pl{c}")
                        for c in range(C)
                    ]
                    if halves:
                        srh = lambda c, hh: inp[
                            i, c, 256 * hh : 256 * (hh + 1)
                        ].rearrange("(j p) w -> p j w", p=128)
                        for hh in range(2):
                            sl = slice(2 * hh, 2 * (hh + 1))
                            for c in range(C):
                                nc.gpsimd.dma_start(pb[c][:, sl], srh(c, hh))
                            nc.vector.tensor_tensor(
                                interior[:, sl], pb[0][:, sl], pb[1][:, sl],
                                op=MIN)
                            nc.vector.tensor_tensor(
                                interior[:, sl], interior[:, sl],
                                pb[2][:, sl], op=MIN)
                    else:
                        for c in range(C):
                            nc.gpsimd.dma_start(pb[c][:], src(c))
                        nc.vector.tensor_tensor(interior, pb[0][:], pb[1][:],
                                                op=MIN)
                        nc.vector.tensor_tensor(interior, interior, pb[2][:],
                                                op=MIN)
                elif i in two_chain:
                    # 2-link accum chain + parallel third channel; one DVE
                    # merge.  Lower latency than the 3-link chain.
                    a = plp.tile([128, NJ, W], FI, tag="pb0", name="tc_a")
                    b = plp.tile([128, NJ, W], FI, tag="pb1", name="tc_b")
                    nc.gpsimd.dma_start(a[:], src(0))
                    nc.gpsimd.dma_start(b[:], src(2))
                    nc.gpsimd.dma_start(a[:], src(1), accum_op=MIN)
                    nc.vector.tensor_tensor(interior, a[:], b[:], op=MIN)
                else:
                    for c in range(C):
                        nc.gpsimd.dma_start(
                            interior, src(c),
                            accum_op=mybir.AluOpType.bypass if c == 0 else MIN,
                        )
                # horizontal filter
                return dyadic(resw, xpad, NJ,
                              off={s for (ii, s) in off_h if ii == i},
                              split_last=h_split)

            def stage2(i, r):
                # transpose to column layout; one PSUM bank + one ACT evac
                # per 128-column block.
                vb = work.tile([128, NB, PITCH], FI, tag="vb", name="vb")
                nc.gpsimd.memset(vb[:, :, 0:L], PADV)
                nc.gpsimd.memset(vb[:, :, L + H : PITCH], PADV)
                for b in range(NB):
                    pt = psum.tile([128, NJ, 128], FI, tag="pt", name="pt")
                    for j in range(NJ):
                        nc.tensor.transpose(
                            pt[:, j, :], r[:, j, 128 * b : 128 * (b + 1)],
                            ident[:],
                        )
                    src_f = pt[:, :, :].rearrange("p n w -> p (n w)")
                    if (i, b) in evac_dve:
                        nc.vector.tensor_copy(vb[:, b, L : L + H], src_f)
                    else:
                        nc.scalar.copy(vb[:, b, L : L + H], src_f)

                # vertical filter per column-block pair
                offv = {s for (ii, s) in off_v if ii == i}
                u_pairs = [
                    dyadic(resw, vb[:, 2 * bp : 2 * (bp + 1), :], 2, off=offv,
                           split_last=v_split)
                    for bp in range(2)
                ]

                # transpose back, f32 out, store per row-tile
                o = opool.tile([128, NJ, W], F32, name="o")
                for j in range(NJ):
                    pt = psum.tile([128, NB, 128], FI, tag="pt", name="pt2")
                    for b in range(NB):
                        nc.tensor.transpose(
                            pt[:, b, :],
                            u_pairs[b // 2][:, b % 2, 128 * j : 128 * (j + 1)],
                            ident[:],
                        )
                    nc.scalar.copy(
                        o[:, j, :],
                        pt[:, :, :].rearrange("p n w -> p (n w)"),
                    )
                    (nc.sync if out_eng == "sync" else nc.scalar).dma_start(
                        out[i, 0, 128 * j : 128 * (j + 1)].rearrange(
                            "(q p) w -> p q w", p=128
                        ),
                        o[:, j : j + 1, :],
                    )

            for i in range(IMGS):
                stage2(i, stage1(i))

    if not split_waits:
        return nc
    import concourse.mybir as mybir
    nsplit = 0
    for bb in nc.main_func.blocks:
        idx = 0
        while idx < len(bb.instructions):
            ins = bb.instructions[idx]
            si = ins.sync_info
            if si is not None and si.on_wait and len(si.on_wait) > 1:
                waits = list(si.on_wait)
                for w in waits[:-1]:
                    nop = mybir.InstNoOp(
                        name=f"W-split-{nsplit}", ins=[], outs=[]
                    )
                    nop.engine = ins.engine
                    nop.sync_info = mybir.SyncInfo(on_wait=[w], on_update=[])
                    bb.instructions.insert(idx, nop)
                    nsplit += 1
                    idx += 1
                ins.sync_info = mybir.SyncInfo(
                    on_wait=[waits[-1]], on_update=list(si.on_update or [])
                )
            idx += 1
    return nc


def _get_nc():
    if "nc" not in _cache:
        _cache["nc"] = _build_nc()
    return _cache["nc"]


def kernel(I, k):
    from concourse.bass_utils import run_bass_kernel_spmd

    k = int(np.asarray(k))
    assert k == K, f"kernel compiled for k={K}, got {k}"
    I = np.ascontiguousarray(np.asarray(I), dtype=np.float32)
    B = I.shape[0]
    assert I.shape == (B, C, H, W) and B == N_CORES * IMGS

    nc = _get_nc()
    in_maps = [
        {"inp": I[c * IMGS : (c + 1) * IMGS]} for c in range(N_CORES)
    ]
    res = run_bass_kernel_spmd(nc, in_maps, list(range(N_CORES))).results
    return np.concatenate([res[c]["out"] for c in range(N_CORES)], axis=0)
